# revision 6
# baseline (speedup 1.0000x reference)
"""Trainium2 Bass kernel for nn_BandwidthConstrainedComm.

GNN message passing: per batch element, N=256 agents each generate a
message (MLP -> compress -> decompress), compute pairwise bilinear
relevance scores, pick top-K=8 senders (softmax gated), aggregate their
messages, and run a receiver MLP over [obs, agg].

Sharding: pure data parallel over batch B=128 -> 16 per core x 8 cores.

v2 design notes:
  - all inputs pre-cast to bf16 on the host; obs staged as [D, bpc, N]
    so every DMA line is a dense 1KB segment and there are no on-chip
    casts of obs at all.
  - W2@Wc@Wd fused into one [H1, MSG] matrix on the host (the message
    path between the two nonlinearities is purely linear).
  - message bias bf = b2@Wc@Wd + bc@Wd + bd is folded THROUGH the
    aggregation (softmax gates sum to 1 => agg = agg_nb + bf) into the
    receiver matmul via a ones-row in aggT and a host-precomputed
    bf@Wr1c row appended to Wr1c.
  - br2 is added on the host after gathering (final linear bias).
  - gate normalization folded into the PE transpose: transpose(U, D)
    computes U.T @ D; with D = diag(1/den) this transposes AND
    normalizes in one matmul. diag tiles built on gpsimd (idle engine).
  - output written as bf16 [D, bpc, N] (dense lines), un-transposed and
    f32-cast on the host.
  - top-8 via DVE Max8 over exp'd scores; (E >= t8)*E via one
    scalar_tensor_tensor whose accum_out gives the softmax denominator.
  - softmax without max subtraction (scores bounded ~30 -> exp finite).
"""

import sys

sys.path.insert(0, "/opt/trn_rl_repo")

import numpy as np

# problem dims (hardcoded per contract)
B, N, D = 128, 256, 256
MSG, CD, K = 64, 32, 8
H1, H2 = 128, 256
NCORES = 8
BPC = B // NCORES  # batches per core

_CACHE = {}


def build_program(bpc=BPC, passes=1):
    import concourse.bacc as bacc
    import concourse.mybir as mybir
    import concourse.tile as tile
    from concourse.masks import make_identity
    from contextlib import ExitStack

    dt = mybir.dt
    f32, bf16 = dt.float32, dt.bfloat16
    AF = mybir.ActivationFunctionType
    OP = mybir.AluOpType

    assert bpc % 2 == 0
    npairs = bpc // 2

    nc = bacc.Bacc("TRN2", target_bir_lowering=False, debug=False,
                   num_devices=NCORES)

    obsT_d = nc.dram_tensor("obsT", [D, bpc, N], bf16, kind="ExternalInput")
    W1_d = nc.dram_tensor("W1", [D, H1], bf16, kind="ExternalInput")
    Wf_d = nc.dram_tensor("Wf", [H1, MSG], bf16, kind="ExternalInput")
    Wbil_d = nc.dram_tensor("Wbil", [D, D], bf16, kind="ExternalInput")
    Wr1a_d = nc.dram_tensor("Wr1a", [D, H2], bf16, kind="ExternalInput")
    Wr1c_d = nc.dram_tensor("Wr1c", [MSG + 1, H2], bf16,
                            kind="ExternalInput")
    Wr2_d = nc.dram_tensor("Wr2", [H2, D], bf16, kind="ExternalInput")
    b1_d = nc.dram_tensor("b1", [H1], f32, kind="ExternalInput")
    br1_d = nc.dram_tensor("br1", [H2], f32, kind="ExternalInput")
    out_d = nc.dram_tensor("out", [D, bpc, N], bf16, kind="ExternalOutput")

    with tile.TileContext(nc) as tc, ExitStack() as ctx:
        wp = ctx.enter_context(tc.tile_pool(name="wp", bufs=1))
        dp = ctx.enter_context(tc.tile_pool(name="dp", bufs=3))
        sp = ctx.enter_context(tc.tile_pool(name="sp", bufs=3))
        pp = ctx.enter_context(tc.tile_pool(name="pp", bufs=1, space="PSUM"))

        # PSUM banks (8 x 2KB/partition):
        #   w2k  : hT, s_b0, s_b1, outT0, outT1  (2KB tiles, bufs=3)
        #   tmp  : [128, 2, 512] f32 (4KB, bufs=1)
        #   rps  : rps0/rps1 + hT? -> [128,512] f32 bufs=2
        #   agg  : aggT [64, 512] f32 bufs=1
        #   gtm  : Gt bf16 1KB bufs=1 + msn bf16 256B bufs=2

        # ---------------- one-time setup ----------------
        ident = wp.tile([128, 128], f32)
        make_identity(nc, ident[:])
        ident_b = wp.tile([128, 128], bf16)
        nc.vector.tensor_copy(ident_b[:], ident[:])
        warm_ps = pp.tile([128, 128], f32, tag="w2k", bufs=3)
        nc.tensor.transpose(warm_ps[:], ident[:], ident[:])

        def loadw(dram_ap, shape, name, eng=nc.sync):
            t = wp.tile(shape, bf16, name=name)
            eng.dma_start(t[:], dram_ap)
            return t

        W1_r0 = loadw(W1_d[0:128, :], [128, H1], "W1a")
        W1_r1 = loadw(W1_d[128:256, :], [128, H1], "W1b", nc.gpsimd)
        Wf_b = loadw(Wf_d[:], [H1, MSG], "Wf")
        Wb_r0 = loadw(Wbil_d[0:128, :], [128, D], "Wba", nc.gpsimd)
        Wb_r1 = loadw(Wbil_d[128:256, :], [128, D], "Wbb")
        Wr1_r0 = loadw(Wr1a_d[0:128, :], [128, H2], "Wr1a", nc.gpsimd)
        Wr1_r1 = loadw(Wr1a_d[128:256, :], [128, H2], "Wr1b")
        Wr1c_b = loadw(Wr1c_d[:], [MSG + 1, H2], "Wr1c", nc.gpsimd)
        Wr2_r0 = loadw(Wr2_d[0:128, :], [128, D], "Wr2a")
        Wr2_r1 = loadw(Wr2_d[128:256, :], [128, D], "Wr2b", nc.gpsimd)

        def load_bias(dram, p, name, off=0):
            t = wp.tile([p, 1], f32, name=name)
            nc.sync.dma_start(
                t[:], dram[off:off + p].rearrange("(p o) -> p o", o=1))
            return t

        b1_sb = load_bias(b1_d, H1, "b1s")
        br1_sb0 = load_bias(br1_d, 128, "br1s0")
        br1_sb1 = load_bias(br1_d, 128, "br1s1", off=128)

        # persistent aggT tiles with a constant ones-row (row MSG) for
        # the folded message bias
        aggT_tiles = []
        for i in range(2):
            t = wp.tile([MSG + 1, 2 * N], bf16, name=f"aggTp{i}")
            nc.vector.memset(t[MSG:MSG + 1, :], 1.0)
            aggT_tiles.append(t)

        # ---------------- main loop over batch pairs ----------------
        for _ in range(passes):
            for p in range(npairs):
                b0 = 2 * p
                od_b = []
                for dc in range(2):
                    ob = dp.tile([128, 2, N], bf16, name=f"od{dc}",
                                 tag=f"od{dc}")
                    (nc.sync if dc == 0 else nc.gpsimd).dma_start(
                        ob[:], obsT_d[128 * dc:128 * (dc + 1),
                                      b0:b0 + 2, :])
                    od_b.append(ob[:].rearrange("d b n -> d (b n)"))

                # ---- message MLP layer 1 ----
                hT_ps = pp.tile([H1, 2 * N], f32, tag="rps", bufs=2)
                nc.tensor.matmul(hT_ps[:], W1_r0[:], od_b[0],
                                 start=True, stop=False)
                nc.tensor.matmul(hT_ps[:], W1_r1[:], od_b[1],
                                 start=False, stop=True)
                hT_b = sp.tile([H1, 2 * N], bf16, name="hT_b")
                nc.scalar.activation(hT_b[:], hT_ps[:], AF.Relu,
                                     bias=b1_sb[:])

                # ---- bilinear tmp (pair-wide) ----
                tmp_ps = pp.tile([128, 2, 2 * N], f32, tag="tmp", bufs=1)
                for ec in range(2):
                    nc.tensor.matmul(tmp_ps[:, ec, :],
                                     Wb_r0[:, 128 * ec:128 * (ec + 1)],
                                     od_b[0], start=True, stop=False)
                    nc.tensor.matmul(tmp_ps[:, ec, :],
                                     Wb_r1[:, 128 * ec:128 * (ec + 1)],
                                     od_b[1], start=False, stop=True)
                tmpT_r = sp.tile([128, 2, 2 * N], bf16, name="tmpT_r")
                nc.scalar.activation(
                    tmpT_r[:].rearrange("e c f -> e (c f)"),
                    tmp_ps[:].rearrange("e c f -> e (c f)"), AF.Copy)

                # ---- messages directly in [sender, msg] layout ----
                msgs_b = []
                for bi in range(2):
                    boff = bi * N
                    msn_ps = pp.tile([128, 2, MSG], f32, tag="w2k",
                                     bufs=3, name="msn_ps")
                    for jc in range(2):
                        nc.tensor.matmul(
                            msn_ps[:, jc, :],
                            hT_b[:, boff + 128 * jc:boff + 128 * (jc + 1)],
                            Wf_b[:], start=True, stop=True)
                    mb = sp.tile([128, 2, MSG], bf16, name="msgs_b", bufs=3)
                    nc.vector.tensor_copy(mb[:], msn_ps[:])
                    msgs_b.append(mb)

                # ---- scores for both batches ----
                s_ps = []
                for bi in range(2):
                    boff = bi * N
                    sp_t = pp.tile([128, 2, N], f32, tag="w2k", bufs=3,
                                   name=f"s{bi}_ps")
                    for ic in range(2):
                        ioff = boff + 128 * ic
                        nc.tensor.matmul(sp_t[:, ic, :],
                                         tmpT_r[:, 0, ioff:ioff + 128],
                                         od_b[0][:, boff:boff + N],
                                         start=True, stop=False)
                        nc.tensor.matmul(sp_t[:, ic, :],
                                         tmpT_r[:, 1, ioff:ioff + 128],
                                         od_b[1][:, boff:boff + N],
                                         start=False, stop=True)
                    s_ps.append(sp_t)

                # receiver-MLP obs contributions hoisted ahead of gating
                rps_t = []
                for mi in range(2):
                    rps = pp.tile([128, 2 * N], f32, tag="rps", bufs=2,
                                  name=f"r{mi}_ps")
                    ms = 128 * mi
                    nc.tensor.matmul(rps[:], Wr1_r0[:, ms:ms + 128],
                                     od_b[0], start=True, stop=False)
                    nc.tensor.matmul(rps[:], Wr1_r1[:, ms:ms + 128],
                                     od_b[1], start=False, stop=False)
                    rps_t.append(rps)

                # ---- gating + aggregation per batch ----
                aggT_ps = pp.tile([MSG, 2 * N], f32, tag="agg", bufs=1)
                for bi in range(2):
                    boff = bi * N
                    E = sp.tile([128, 2, N], bf16, name="E", bufs=3)
                    nc.scalar.activation(
                        E[:].rearrange("p c f -> p (c f)"),
                        s_ps[bi][:].rearrange("p c f -> p (c f)"), AF.Exp)

                    Gt_ps = pp.tile([128, 2, N], bf16, tag="tmp", bufs=1,
                                    name="Gt_ps")
                    den = sp.tile([128, 2], f32, name="den", bufs=4)
                    rden = sp.tile([128, 2], f32, name="rden", bufs=4)
                    U = sp.tile([128, 2, N], bf16, name="U", bufs=3)
                    for ic in range(2):
                        top8 = sp.tile([128, 8], bf16, name="top8", bufs=4)
                        nc.vector.max(out=top8[:], in_=E[:, ic, :])
                        nc.vector.scalar_tensor_tensor(
                            out=U[:, ic, :], in0=E[:, ic, :],
                            scalar=top8[:, 7:8], in1=E[:, ic, :],
                            op0=OP.is_ge, op1=OP.mult,
                            accum_out=den[:, ic:ic + 1])
                    nc.vector.reciprocal(rden[:], den[:])
                    G = sp.tile([128, 2, N], bf16, name="G", bufs=3)
                    for ic in range(2):
                        nc.gpsimd.tensor_scalar_mul(G[:, ic, :],
                                                    U[:, ic, :],
                                                    rden[:, ic:ic + 1])
                        for jc in range(2):
                            nc.tensor.transpose(
                                Gt_ps[:, jc, 128 * ic:128 * (ic + 1)],
                                G[:, ic, 128 * jc:128 * (jc + 1)],
                                ident_b[:])

                    Gt_b = sp.tile([128, 2, N], bf16, name="Gt_b", bufs=3)
                    nc.vector.tensor_copy(Gt_b[:], Gt_ps[:])

                    # aggT[m, i] = sum_j msgs[j, m] * Gt[j, i]
                    nc.tensor.matmul(aggT_ps[:, boff:boff + N],
                                     msgs_b[bi][:, 0, :], Gt_b[:, 0, :],
                                     start=True, stop=False)
                    nc.tensor.matmul(aggT_ps[:, boff:boff + N],
                                     msgs_b[bi][:, 1, :], Gt_b[:, 1, :],
                                     start=False, stop=True)

                aggT_r = aggT_tiles[p % 2]
                nc.vector.tensor_copy(aggT_r[0:MSG, :], aggT_ps[:])

                # ---- receiver MLP: close groups with the agg term ----
                rT_r = []
                for mi in range(2):
                    rps = rps_t[mi]
                    ms = 128 * mi
                    nc.tensor.matmul(rps[:], Wr1c_b[:, ms:ms + 128],
                                     aggT_r[:], start=False, stop=True)
                    rr = sp.tile([128, 2 * N], bf16, name=f"r{mi}_r",
                                 tag=f"r{mi}r")
                    nc.scalar.activation(
                        rr[:], rps[:], AF.Relu,
                        bias=(br1_sb0 if mi == 0 else br1_sb1)[:])
                    rT_r.append(rr)

                # ---- output in [d, (b n)] layout ----
                for dc in range(2):
                    ds = 128 * dc
                    o_ps = pp.tile([128, 2 * N], f32, tag="w2k", bufs=3,
                                   name="o_ps")
                    nc.tensor.matmul(o_ps[:], Wr2_r0[:, ds:ds + 128],
                                     rT_r[0][:], start=True, stop=False)
                    nc.tensor.matmul(o_ps[:], Wr2_r1[:, ds:ds + 128],
                                     rT_r[1][:], start=False, stop=True)
                    o_sb = dp.tile([128, 2, N], bf16, name=f"o_sb{dc}",
                                   tag=f"osb{dc}")
                    nc.vector.tensor_copy(
                        o_sb[:].rearrange("d b n -> d (b n)"), o_ps[:])
                    (nc.sync if dc == 0 else nc.gpsimd).dma_start(
                        out_d[ds:ds + 128, b0:b0 + 2, :], o_sb[:])

    nc.compile()
    return nc


def _np_inputs_for_core(inputs, core, bpc=BPC):
    import ml_dtypes

    bf = ml_dtypes.bfloat16
    obs = np.asarray(inputs["obs_all"], np.float32)
    lo = core * bpc
    obsT = np.ascontiguousarray(
        obs[lo:lo + bpc].transpose(2, 0, 1)).astype(bf)

    W1 = np.asarray(inputs["W1"], np.float32)
    W2 = np.asarray(inputs["W2"], np.float32)
    b2 = np.asarray(inputs["b2"], np.float32)
    Wc = np.asarray(inputs["Wc"], np.float32)
    bc = np.asarray(inputs["bc"], np.float32)
    Wd = np.asarray(inputs["Wd"], np.float32)
    bd = np.asarray(inputs["bd"], np.float32)
    Wr1 = np.asarray(inputs["Wr1"], np.float32)

    Wf = (W2 @ Wc) @ Wd                              # [H1, MSG]
    bf_vec = (b2 @ Wc) @ Wd + bc @ Wd + bd           # [MSG]
    Wr1c = Wr1[D:D + MSG]                            # [MSG, H2]
    Wr1c_aug = np.vstack([Wr1c, (bf_vec @ Wr1c)[None, :]])  # [MSG+1, H2]

    return {
        "obsT": obsT,
        "W1": W1.astype(bf),
        "Wf": Wf.astype(bf),
        "Wbil": np.asarray(inputs["Wbil"], np.float32).astype(bf),
        "Wr1a": Wr1[0:D].astype(bf),
        "Wr1c": np.ascontiguousarray(Wr1c_aug).astype(bf),
        "Wr2": np.asarray(inputs["Wr2"], np.float32).astype(bf),
        "b1": np.asarray(inputs["b1"], np.float32),
        "br1": np.asarray(inputs["br1"], np.float32),
    }


def _finish(outT, br2):
    # outT: [D, bpc, N] bf16 -> [bpc, N, D] f32 + br2
    return outT.astype(np.float32).transpose(1, 2, 0) + br2[None, None, :]


def kernel(**inputs):
    from concourse.bass_utils import run_bass_kernel_spmd

    if "prog" not in _CACHE:
        _CACHE["prog"] = build_program(BPC)
    nc = _CACHE["prog"]

    br2 = np.asarray(inputs["br2"], np.float32)
    core_ids = list(range(NCORES))
    in_maps = [_np_inputs_for_core(inputs, c) for c in core_ids]
    res = run_bass_kernel_spmd(nc, in_maps, core_ids)
    out = np.concatenate(
        [_finish(np.asarray(res.results[c]["out"]), br2)
         for c in core_ids], axis=0)
    return out.astype(np.float32)


# revision 7
# speedup vs baseline: 2.0790x; 2.0790x over previous
"""Trainium2 Bass kernel for nn_BandwidthConstrainedComm.

GNN message passing: per batch element, N=256 agents each generate a
message (MLP -> compress -> decompress), compute pairwise bilinear
relevance scores, pick top-K=8 senders (softmax gated), aggregate their
messages, and run a receiver MLP over [obs, agg].

Sharding: pure data parallel over batch B=128 -> 16 per core x 8 cores.

v2 design notes:
  - all inputs pre-cast to bf16 on the host; obs staged as [D, bpc, N]
    so every DMA line is a dense 1KB segment and there are no on-chip
    casts of obs at all.
  - W2@Wc@Wd fused into one [H1, MSG] matrix on the host (the message
    path between the two nonlinearities is purely linear).
  - message bias bf = b2@Wc@Wd + bc@Wd + bd is folded THROUGH the
    aggregation (softmax gates sum to 1 => agg = agg_nb + bf) into the
    receiver matmul via a ones-row in aggT and a host-precomputed
    bf@Wr1c row appended to Wr1c.
  - br2 is added on the host after gathering (final linear bias).
  - gate normalization folded into the PE transpose: transpose(U, D)
    computes U.T @ D; with D = diag(1/den) this transposes AND
    normalizes in one matmul. diag tiles built on gpsimd (idle engine).
  - output written as bf16 [D, bpc, N] (dense lines), un-transposed and
    f32-cast on the host.
  - top-8 via DVE Max8 over exp'd scores; (E >= t8)*E via one
    scalar_tensor_tensor whose accum_out gives the softmax denominator.
  - softmax without max subtraction (scores bounded ~30 -> exp finite).
"""

import sys

sys.path.insert(0, "/opt/trn_rl_repo")

import numpy as np

# problem dims (hardcoded per contract)
B, N, D = 128, 256, 256
MSG, CD, K = 64, 32, 8
H1, H2 = 128, 256
NCORES = 8
BPC = B // NCORES  # batches per core

_CACHE = {}


def build_program(bpc=BPC, passes=1):
    import concourse.bacc as bacc
    import concourse.mybir as mybir
    import concourse.tile as tile
    from concourse.masks import make_identity
    from contextlib import ExitStack

    dt = mybir.dt
    f32, bf16 = dt.float32, dt.bfloat16
    AF = mybir.ActivationFunctionType
    OP = mybir.AluOpType

    assert bpc % 2 == 0
    npairs = bpc // 2

    nc = bacc.Bacc("TRN2", target_bir_lowering=False, debug=False,
                   num_devices=NCORES)

    obsT_d = nc.dram_tensor("obsT", [D, bpc, N], bf16, kind="ExternalInput")
    W1_d = nc.dram_tensor("W1", [D, H1], bf16, kind="ExternalInput")
    Wf_d = nc.dram_tensor("Wf", [H1, MSG], bf16, kind="ExternalInput")
    Wbil_d = nc.dram_tensor("Wbil", [D, D], bf16, kind="ExternalInput")
    Wr1a_d = nc.dram_tensor("Wr1a", [D, H2], bf16, kind="ExternalInput")
    Wr1c_d = nc.dram_tensor("Wr1c", [MSG + 1, H2], bf16,
                            kind="ExternalInput")
    Wr2_d = nc.dram_tensor("Wr2", [H2, D], bf16, kind="ExternalInput")
    b1_d = nc.dram_tensor("b1", [H1], f32, kind="ExternalInput")
    br1_d = nc.dram_tensor("br1", [H2], f32, kind="ExternalInput")
    out_d = nc.dram_tensor("out", [D, bpc, N], bf16, kind="ExternalOutput")

    with tile.TileContext(nc) as tc, ExitStack() as ctx:
        wp = ctx.enter_context(tc.tile_pool(name="wp", bufs=1))
        dp = ctx.enter_context(tc.tile_pool(name="dp", bufs=3))
        sp = ctx.enter_context(tc.tile_pool(name="sp", bufs=3))
        pp = ctx.enter_context(tc.tile_pool(name="pp", bufs=1, space="PSUM"))

        # PSUM banks (8 x 2KB/partition):
        #   w2k  : hT, s_b0, s_b1, outT0, outT1  (2KB tiles, bufs=3)
        #   tmp  : [128, 2, 512] f32 (4KB, bufs=1)
        #   rps  : rps0/rps1 + hT? -> [128,512] f32 bufs=2
        #   agg  : aggT [64, 512] f32 bufs=1
        #   gtm  : Gt bf16 1KB bufs=1 + msn bf16 256B bufs=2

        # ---------------- one-time setup ----------------
        ident = wp.tile([128, 128], f32)
        make_identity(nc, ident[:])
        ident_b = wp.tile([128, 128], bf16)
        nc.vector.tensor_copy(ident_b[:], ident[:])
        warm_ps = pp.tile([128, 128], f32, tag="w2k", bufs=3)
        nc.tensor.transpose(warm_ps[:], ident[:], ident[:])

        def loadw(dram_ap, shape, name, eng=nc.sync):
            t = wp.tile(shape, bf16, name=name)
            eng.dma_start(t[:], dram_ap)
            return t

        W1_r0 = loadw(W1_d[0:128, :], [128, H1], "W1a")
        W1_r1 = loadw(W1_d[128:256, :], [128, H1], "W1b", nc.gpsimd)
        Wf_b = loadw(Wf_d[:], [H1, MSG], "Wf")
        Wb_r0 = loadw(Wbil_d[0:128, :], [128, D], "Wba", nc.gpsimd)
        Wb_r1 = loadw(Wbil_d[128:256, :], [128, D], "Wbb")
        Wr1_r0 = loadw(Wr1a_d[0:128, :], [128, H2], "Wr1a", nc.gpsimd)
        Wr1_r1 = loadw(Wr1a_d[128:256, :], [128, H2], "Wr1b")
        Wr1c_b = loadw(Wr1c_d[:], [MSG + 1, H2], "Wr1c", nc.gpsimd)
        Wr2_r0 = loadw(Wr2_d[0:128, :], [128, D], "Wr2a")
        Wr2_r1 = loadw(Wr2_d[128:256, :], [128, D], "Wr2b", nc.gpsimd)

        def load_bias(dram, p, name, off=0):
            t = wp.tile([p, 1], f32, name=name)
            nc.sync.dma_start(
                t[:], dram[off:off + p].rearrange("(p o) -> p o", o=1))
            return t

        b1_sb = load_bias(b1_d, H1, "b1s")
        br1_sb0 = load_bias(br1_d, 128, "br1s0")
        br1_sb1 = load_bias(br1_d, 128, "br1s1", off=128)

        # persistent aggT tiles with a constant ones-row (row MSG) for
        # the folded message bias
        aggT_tiles = []
        for i in range(2):
            t = wp.tile([MSG + 1, 2 * N], bf16, name=f"aggTp{i}")
            nc.vector.memset(t[MSG:MSG + 1, :], 1.0)
            aggT_tiles.append(t)

        # ---------------- main loop over batch pairs ----------------
        for _ in range(passes):
            for p in range(npairs):
                b0 = 2 * p
                od_b = []
                for dc in range(2):
                    ob = dp.tile([128, 2, N], bf16, name=f"od{dc}",
                                 tag=f"od{dc}")
                    nc.sync.dma_start(
                        ob[:], obsT_d[128 * dc:128 * (dc + 1),
                                      b0:b0 + 2, :])
                    od_b.append(ob[:].rearrange("d b n -> d (b n)"))

                # ---- message MLP layer 1 ----
                hT_ps = pp.tile([H1, 2 * N], f32, tag="w2k", bufs=3)
                nc.tensor.matmul(hT_ps[:], W1_r0[:], od_b[0],
                                 start=True, stop=False)
                nc.tensor.matmul(hT_ps[:], W1_r1[:], od_b[1],
                                 start=False, stop=True)
                hT_b = sp.tile([H1, 2 * N], bf16, name="hT_b")
                nc.scalar.activation(hT_b[:], hT_ps[:], AF.Relu,
                                     bias=b1_sb[:])

                # ---- bilinear tmp (pair-wide) ----
                tmp_ps = pp.tile([128, 2, 2 * N], f32, tag="tmp", bufs=1)
                for ec in range(2):
                    nc.tensor.matmul(tmp_ps[:, ec, :],
                                     Wb_r0[:, 128 * ec:128 * (ec + 1)],
                                     od_b[0], start=True, stop=False)
                    nc.tensor.matmul(tmp_ps[:, ec, :],
                                     Wb_r1[:, 128 * ec:128 * (ec + 1)],
                                     od_b[1], start=False, stop=True)
                tmpT_r = sp.tile([128, 2, 2 * N], bf16, name="tmpT_r")
                nc.scalar.activation(
                    tmpT_r[:].rearrange("e c f -> e (c f)"),
                    tmp_ps[:].rearrange("e c f -> e (c f)"), AF.Copy)

                # ---- messages directly in [sender, msg] layout ----
                msn_ps = pp.tile([128, 4, MSG], f32, tag="w2k",
                                 bufs=3, name="msn_ps")
                for q in range(4):
                    nc.tensor.matmul(
                        msn_ps[:, q, :],
                        hT_b[:, 128 * q:128 * (q + 1)],
                        Wf_b[:], start=True, stop=True)
                msgs_b = sp.tile([128, 4, MSG], bf16, name="msgs_b")
                nc.vector.tensor_copy(msgs_b[:], msn_ps[:])

                # ---- scores for both batches ----
                s_ps = []
                for bi in range(2):
                    boff = bi * N
                    sp_t = pp.tile([128, 2, N], f32, tag="w2k", bufs=3,
                                   name=f"s{bi}_ps")
                    for ic in range(2):
                        ioff = boff + 128 * ic
                        nc.tensor.matmul(sp_t[:, ic, :],
                                         tmpT_r[:, 0, ioff:ioff + 128],
                                         od_b[0][:, boff:boff + N],
                                         start=True, stop=False)
                        nc.tensor.matmul(sp_t[:, ic, :],
                                         tmpT_r[:, 1, ioff:ioff + 128],
                                         od_b[1][:, boff:boff + N],
                                         start=False, stop=True)
                    s_ps.append(sp_t)

                # receiver-MLP obs contributions hoisted ahead of gating
                rps_t = []
                for mi in range(2):
                    rps = pp.tile([128, 2 * N], f32, tag="rps", bufs=2,
                                  name=f"r{mi}_ps")
                    ms = 128 * mi
                    nc.tensor.matmul(rps[:], Wr1_r0[:, ms:ms + 128],
                                     od_b[0], start=True, stop=False)
                    nc.tensor.matmul(rps[:], Wr1_r1[:, ms:ms + 128],
                                     od_b[1], start=False, stop=False)
                    rps_t.append(rps)

                # ---- gating + aggregation per batch ----
                aggT_ps = pp.tile([MSG, 2 * N], f32, tag="agg", bufs=1)
                for bi in range(2):
                    boff = bi * N
                    E = sp.tile([128, 2, N], bf16, name="E", bufs=3)
                    nc.scalar.activation(
                        E[:].rearrange("p c f -> p (c f)"),
                        s_ps[bi][:].rearrange("p c f -> p (c f)"), AF.Exp)

                    Gt_ps = pp.tile([128, 2, N], f32, tag="tmp", bufs=1,
                                    name="Gt_ps")
                    den = sp.tile([128, 2], f32, name="den", bufs=4)
                    rden = sp.tile([128, 2], f32, name="rden", bufs=4)
                    U = sp.tile([128, 2, N], bf16, name="U", bufs=3)
                    for ic in range(2):
                        top8 = sp.tile([128, 8], bf16, name="top8", bufs=4)
                        nc.vector.max(out=top8[:], in_=E[:, ic, :])
                        nc.vector.scalar_tensor_tensor(
                            out=U[:, ic, :], in0=E[:, ic, :],
                            scalar=top8[:, 7:8], in1=E[:, ic, :],
                            op0=OP.is_ge, op1=OP.mult,
                            accum_out=den[:, ic:ic + 1])
                    nc.vector.reciprocal(rden[:], den[:])
                    for ic in range(2):
                        dg = sp.tile([128, 128], bf16, name="diag", bufs=4)
                        nc.vector.tensor_scalar_mul(dg[:], ident_b[:],
                                                    rden[:, ic:ic + 1])
                        # Gt[j, i] = sum_k U[k, j] * diag[k, i]
                        #          = U[i, j] * rden[i]  (transpose+normalize)
                        for jc in range(2):
                            nc.tensor.matmul(
                                Gt_ps[:, jc, 128 * ic:128 * (ic + 1)],
                                U[:, ic, 128 * jc:128 * (jc + 1)],
                                dg[:], start=True, stop=True)

                    Gt_b = sp.tile([128, 2, N], bf16, name="Gt_b", bufs=3)
                    nc.vector.tensor_copy(Gt_b[:], Gt_ps[:])

                    # aggT[m, i] = sum_j msgs[j, m] * Gt[j, i]
                    nc.tensor.matmul(aggT_ps[:, boff:boff + N],
                                     msgs_b[:, 2 * bi, :], Gt_b[:, 0, :],
                                     start=True, stop=False)
                    nc.tensor.matmul(aggT_ps[:, boff:boff + N],
                                     msgs_b[:, 2 * bi + 1, :], Gt_b[:, 1, :],
                                     start=False, stop=True)

                aggT_r = aggT_tiles[p % 2]
                nc.vector.tensor_copy(aggT_r[0:MSG, :], aggT_ps[:])

                # ---- receiver MLP: close groups with the agg term ----
                rT_r = []
                for mi in range(2):
                    rps = rps_t[mi]
                    ms = 128 * mi
                    nc.tensor.matmul(rps[:], Wr1c_b[:, ms:ms + 128],
                                     aggT_r[:], start=False, stop=True)
                    rr = sp.tile([128, 2 * N], bf16, name=f"r{mi}_r",
                                 tag=f"r{mi}r")
                    nc.scalar.activation(
                        rr[:], rps[:], AF.Relu,
                        bias=(br1_sb0 if mi == 0 else br1_sb1)[:])
                    rT_r.append(rr)

                # ---- output in [d, (b n)] layout ----
                for dc in range(2):
                    ds = 128 * dc
                    o_ps = pp.tile([128, 2 * N], f32, tag="w2k", bufs=3,
                                   name="o_ps")
                    nc.tensor.matmul(o_ps[:], Wr2_r0[:, ds:ds + 128],
                                     rT_r[0][:], start=True, stop=False)
                    nc.tensor.matmul(o_ps[:], Wr2_r1[:, ds:ds + 128],
                                     rT_r[1][:], start=False, stop=True)
                    o_sb = dp.tile([128, 2, N], bf16, name=f"o_sb{dc}",
                                   tag=f"osb{dc}")
                    nc.scalar.activation(
                        o_sb[:].rearrange("d b n -> d (b n)"), o_ps[:],
                        AF.Copy)
                    nc.sync.dma_start(
                        out_d[ds:ds + 128, b0:b0 + 2, :], o_sb[:])

    nc.compile()
    return nc


def _np_inputs_for_core(inputs, core, bpc=BPC):
    import ml_dtypes

    bf = ml_dtypes.bfloat16
    obs = np.asarray(inputs["obs_all"], np.float32)
    lo = core * bpc
    obsT = np.ascontiguousarray(
        obs[lo:lo + bpc].transpose(2, 0, 1)).astype(bf)

    W1 = np.asarray(inputs["W1"], np.float32)
    W2 = np.asarray(inputs["W2"], np.float32)
    b2 = np.asarray(inputs["b2"], np.float32)
    Wc = np.asarray(inputs["Wc"], np.float32)
    bc = np.asarray(inputs["bc"], np.float32)
    Wd = np.asarray(inputs["Wd"], np.float32)
    bd = np.asarray(inputs["bd"], np.float32)
    Wr1 = np.asarray(inputs["Wr1"], np.float32)

    Wf = (W2 @ Wc) @ Wd                              # [H1, MSG]
    bf_vec = (b2 @ Wc) @ Wd + bc @ Wd + bd           # [MSG]
    Wr1c = Wr1[D:D + MSG]                            # [MSG, H2]
    Wr1c_aug = np.vstack([Wr1c, (bf_vec @ Wr1c)[None, :]])  # [MSG+1, H2]

    return {
        "obsT": obsT,
        "W1": W1.astype(bf),
        "Wf": Wf.astype(bf),
        "Wbil": np.asarray(inputs["Wbil"], np.float32).astype(bf),
        "Wr1a": Wr1[0:D].astype(bf),
        "Wr1c": np.ascontiguousarray(Wr1c_aug).astype(bf),
        "Wr2": np.asarray(inputs["Wr2"], np.float32).astype(bf),
        "b1": np.asarray(inputs["b1"], np.float32),
        "br1": np.asarray(inputs["br1"], np.float32),
    }


def _finish(outT, br2):
    # outT: [D, bpc, N] bf16 -> [bpc, N, D] f32 + br2
    return outT.astype(np.float32).transpose(1, 2, 0) + br2[None, None, :]


def kernel(**inputs):
    from concourse.bass_utils import run_bass_kernel_spmd

    if "prog" not in _CACHE:
        _CACHE["prog"] = build_program(BPC)
    nc = _CACHE["prog"]

    br2 = np.asarray(inputs["br2"], np.float32)
    core_ids = list(range(NCORES))
    in_maps = [_np_inputs_for_core(inputs, c) for c in core_ids]
    res = run_bass_kernel_spmd(nc, in_maps, core_ids)
    out = np.concatenate(
        [_finish(np.asarray(res.results[c]["out"]), br2)
         for c in core_ids], axis=0)
    return out.astype(np.float32)


# revision 15
# speedup vs baseline: 3.3526x; 1.6126x over previous
"""Trainium2 Bass kernel for nn_BandwidthConstrainedComm.

GNN message passing: per batch element, N=256 agents each generate a
message (MLP -> compress -> decompress), compute pairwise bilinear
relevance scores, pick top-K=8 senders (softmax gated), aggregate their
messages, and run a receiver MLP over [obs, agg].

Sharding: pure data parallel over batch B=128 -> 16 per core x 8 cores.

Design notes:
  - all inputs pre-cast to bf16 on the host; obs staged as [D, bpc, N]
    so every DMA line is a dense 1KB segment and there are no on-chip
    casts of obs at all.
  - W2@Wc@Wd fused into one [H1, MSG] matrix on the host (the message
    path between the two nonlinearities is purely linear).
  - message bias bf = b2@Wc@Wd + bc@Wd + bd is folded THROUGH the
    aggregation (softmax gates sum to 1 => agg = agg_nb + bf) into the
    receiver matmul via a ones-row in aggT and a host-precomputed
    bf@Wr1c row appended to Wr1c.
  - br2 is added on the host after gathering (final linear bias).
  - top-8 via DVE Max8 over exp'd scores; U = (E >= t8)*E via one
    scalar_tensor_tensor; den = reduce_sum(top8). softmax without max
    subtraction (scores bounded ~30 -> exp finite).
  - gate transpose+normalize in ONE regular matmul per 128-chunk:
    Gt = U.T @ diag(1/den)  (lhsT=U, rhs=diag built by tensor_scalar
    from a persistent identity).
  - output written as bf16 [D, bpc, N] (dense lines), un-transposed and
    f32-cast on the host.
  - two-stage software pipeline: emit_pre(p+1) [DMA, h/tmp/msgs/score
    matmuls + DVE gating] is queued before emit_mid/out(p) [gating-
    dependent matmuls, receiver MLP, output] so the in-order tensor
    queue never head-of-line blocks on the top-k chain (keeps PE HAM
    warm). PSUM ring tags are sized to exactly 8 banks.
"""

import sys

sys.path.insert(0, "/opt/trn_rl_repo")

import numpy as np

# problem dims (hardcoded per contract)
B, N, D = 128, 256, 256
MSG, CD, K = 64, 32, 8
H1, H2 = 128, 256
NCORES = 8
BPC = B // NCORES  # batches per core

_CACHE = {}


def build_program(bpc=BPC, passes=1):
    import concourse.bacc as bacc
    import concourse.mybir as mybir
    import concourse.tile as tile
    from concourse.masks import make_identity
    from contextlib import ExitStack

    dt = mybir.dt
    f32, bf16 = dt.float32, dt.bfloat16
    AF = mybir.ActivationFunctionType
    OP = mybir.AluOpType

    assert bpc % 2 == 0
    npairs = bpc // 2

    nc = bacc.Bacc("TRN2", target_bir_lowering=False, debug=False,
                   num_devices=NCORES)

    obsT_d = nc.dram_tensor("obsT", [D, bpc, N], bf16, kind="ExternalInput")
    W1_d = nc.dram_tensor("W1", [D, H1], bf16, kind="ExternalInput")
    Wf_d = nc.dram_tensor("Wf", [H1, MSG], bf16, kind="ExternalInput")
    Wbil_d = nc.dram_tensor("Wbil", [D, D], bf16, kind="ExternalInput")
    Wr1a_d = nc.dram_tensor("Wr1a", [D, H2], bf16, kind="ExternalInput")
    Wr1c_d = nc.dram_tensor("Wr1c", [MSG + 1, H2], bf16,
                            kind="ExternalInput")
    Wr2_d = nc.dram_tensor("Wr2", [H2, D], bf16, kind="ExternalInput")
    b1_d = nc.dram_tensor("b1", [H1], f32, kind="ExternalInput")
    br1_d = nc.dram_tensor("br1", [H2], f32, kind="ExternalInput")
    out_d = nc.dram_tensor("out", [D, bpc, N], bf16, kind="ExternalOutput")

    with tile.TileContext(nc) as tc, ExitStack() as ctx:
        wp = ctx.enter_context(tc.tile_pool(name="wp", bufs=1))
        dp = ctx.enter_context(tc.tile_pool(name="dp", bufs=3))
        sp = ctx.enter_context(tc.tile_pool(name="sp", bufs=3))
        pp = ctx.enter_context(tc.tile_pool(name="pp", bufs=1, space="PSUM"))

        # PSUM rings (8 banks x 2KB/partition, exact fit):
        #   w2k (2KB, bufs=4): hT, msn, s_b0, s_b1, o0, o1
        #   tmp (4KB, bufs=1): tmp [128,2,512] f32 / Gt_pair [128,4,256]
        #   rps (2KB, bufs=2): aggT, rps0, rps1

        # ---------------- one-time setup ----------------
        # warmup burst first: dense PE work on a junk tile during the
        # initial DMA/identity latency so the HAM clock-gate is at 8/8
        # when the first real matmuls land. warm_ps lives in the rps
        # ring and is freed by a tiny consumer copy.
        junk = wp.tile([128, 128], bf16, name="junk")
        nc.vector.memset(junk[:], 0.25)
        warm_ps = pp.tile([128, 128], f32, tag="rps", bufs=2)
        for _ in range(16):
            nc.tensor.matmul(warm_ps[:], junk[:], junk[:],
                             start=True, stop=True)
        warm_sink = wp.tile([1, 8], f32, name="warm_sink")
        nc.vector.tensor_copy(warm_sink[:], warm_ps[0:1, 0:8])

        ident = wp.tile([128, 128], f32)
        make_identity(nc, ident[:])
        ident_b = wp.tile([128, 128], bf16)
        nc.vector.tensor_copy(ident_b[:], ident[:])

        def loadw(dram_ap, shape, name, eng=nc.scalar):
            t = wp.tile(shape, bf16, name=name)
            eng.dma_start(t[:], dram_ap)
            return t

        W1_r0 = loadw(W1_d[0:128, :], [128, H1], "W1a")
        W1_r1 = loadw(W1_d[128:256, :], [128, H1], "W1b", nc.gpsimd)
        Wf_b = loadw(Wf_d[:], [H1, MSG], "Wf")
        Wb_r0 = loadw(Wbil_d[0:128, :], [128, D], "Wba", nc.gpsimd)
        Wb_r1 = loadw(Wbil_d[128:256, :], [128, D], "Wbb")
        Wr1_r0 = loadw(Wr1a_d[0:128, :], [128, H2], "Wr1a", nc.gpsimd)
        Wr1_r1 = loadw(Wr1a_d[128:256, :], [128, H2], "Wr1b")
        Wr1c_b = loadw(Wr1c_d[:], [MSG + 1, H2], "Wr1c", nc.gpsimd)
        Wr2_r0 = loadw(Wr2_d[0:128, :], [128, D], "Wr2a")
        Wr2_r1 = loadw(Wr2_d[128:256, :], [128, D], "Wr2b", nc.gpsimd)

        def load_bias(dram, p, name, off=0):
            t = wp.tile([p, 1], f32, name=name)
            nc.scalar.dma_start(
                t[:], dram[off:off + p].rearrange("(p o) -> p o", o=1))
            return t

        b1_sb = load_bias(b1_d, H1, "b1s")
        br1_sb0 = load_bias(br1_d, 128, "br1s0")
        br1_sb1 = load_bias(br1_d, 128, "br1s1", off=128)

        # persistent aggT tiles with a constant ones-row (row MSG) for
        # the folded message bias
        aggT_tiles = []
        for i in range(2):
            t = wp.tile([MSG + 1, 2 * N], bf16, name=f"aggTp{i}")
            nc.vector.memset(t[MSG:MSG + 1, :], 1.0)
            aggT_tiles.append(t)

        # ---------------- main loop over batch pairs ----------------
        # Two-stage software pipeline: emit_pre(p+1) is queued BEFORE
        # emit_post(p) so the in-order tensor queue always has
        # gating-independent matmuls ahead of gating-dependent ones
        # (keeps the PE busy through the DVE top-k chain -> HAM warm).
        state = {}

        def emit_pre(p):
            b0 = 2 * p
            od_b = []
            for dc in range(2):
                ob = dp.tile([128, 2, N], bf16, name=f"od{dc}",
                             tag=f"od{dc}", bufs=3)
                nc.sync.dma_start(
                    ob[:], obsT_d[128 * dc:128 * (dc + 1), b0:b0 + 2, :])
                od_b.append(ob[:].rearrange("d b n -> d (b n)"))

            # message MLP layer 1
            hT_ps = pp.tile([H1, 2 * N], f32, tag="w2k", bufs=4)
            nc.tensor.matmul(hT_ps[:], W1_r0[:], od_b[0],
                             start=True, stop=False)
            nc.tensor.matmul(hT_ps[:], W1_r1[:], od_b[1],
                             start=False, stop=True)
            hT_b = sp.tile([H1, 2 * N], bf16, name="hT_b", bufs=2)
            nc.scalar.activation(hT_b[:], hT_ps[:], AF.Relu, bias=b1_sb[:])

            # bilinear tmp (pair-wide)
            tmp_ps = pp.tile([128, 2, 2 * N], f32, tag="tmp", bufs=1)
            for ec in range(2):
                nc.tensor.matmul(tmp_ps[:, ec, :],
                                 Wb_r0[:, 128 * ec:128 * (ec + 1)],
                                 od_b[0], start=True, stop=False)
                nc.tensor.matmul(tmp_ps[:, ec, :],
                                 Wb_r1[:, 128 * ec:128 * (ec + 1)],
                                 od_b[1], start=False, stop=True)
            tmpT_r = sp.tile([128, 2, 2 * N], bf16, name="tmpT_r", bufs=2)
            nc.scalar.activation(
                tmpT_r[:].rearrange("e c f -> e (c f)"),
                tmp_ps[:].rearrange("e c f -> e (c f)"), AF.Copy)

            # messages directly in [sender, msg] layout
            msn_ps = pp.tile([128, 4, MSG], f32, tag="w2k", bufs=4,
                             name="msn_ps")
            for q in range(4):
                nc.tensor.matmul(msn_ps[:, q, :],
                                 hT_b[:, 128 * q:128 * (q + 1)],
                                 Wf_b[:], start=True, stop=True)
            msgs_b = sp.tile([128, 4, MSG], bf16, name="msgs_b", bufs=3)
            nc.vector.tensor_copy(msgs_b[:], msn_ps[:])

            # scores + DVE gating per batch
            den = sp.tile([128, 4], f32, name="den", bufs=3)
            rden = sp.tile([128, 4], f32, name="rden", bufs=3)
            Us, diags = [], []
            for bi in range(2):
                boff = bi * N
                s_ps = pp.tile([128, 2, N], f32, tag="w2k", bufs=4,
                               name=f"s{bi}_ps")
                for ic in range(2):
                    ioff = boff + 128 * ic
                    nc.tensor.matmul(s_ps[:, ic, :],
                                     tmpT_r[:, 0, ioff:ioff + 128],
                                     od_b[0][:, boff:boff + N],
                                     start=True, stop=False)
                    nc.tensor.matmul(s_ps[:, ic, :],
                                     tmpT_r[:, 1, ioff:ioff + 128],
                                     od_b[1][:, boff:boff + N],
                                     start=False, stop=True)

                E = sp.tile([128, 2, N], bf16, name="E", bufs=3)
                nc.scalar.activation(
                    E[:].rearrange("p c f -> p (c f)"),
                    s_ps[:].rearrange("p c f -> p (c f)"), AF.Exp)

                U = sp.tile([128, 2, N], bf16, name="U", bufs=4)
                for ic in range(2):
                    top8 = sp.tile([128, 8], bf16, name="top8", bufs=4)
                    nc.vector.max(out=top8[:], in_=E[:, ic, :])
                    nc.vector.scalar_tensor_tensor(
                        out=U[:, ic, :], in0=E[:, ic, :],
                        scalar=top8[:, 7:8], in1=E[:, ic, :],
                        op0=OP.is_ge, op1=OP.mult,
                        accum_out=den[:, 2 * bi + ic:2 * bi + ic + 1])
                Us.append(U)

            nc.vector.reciprocal(rden[:], den[:])
            for bi in range(2):
                dgs = []
                for ic in range(2):
                    dg = sp.tile([128, 128], bf16, name="diag", bufs=8)
                    nc.vector.tensor_scalar_mul(dg[:], ident_b[:],
                                                rden[:, 2 * bi + ic:
                                                     2 * bi + ic + 1])
                    dgs.append(dg)
                diags.append(dgs)

            state[p] = (od_b, msgs_b, Us, diags)

        def emit_post(p):
            od_b, msgs_b, Us, diags = state.pop(p)

            aggT_ps = pp.tile([MSG, 2 * N], f32, tag="rps", bufs=2)
            for bi in range(2):
                boff = bi * N
                U, dgs = Us[bi], diags[bi]
                Gt_ps = pp.tile([128, 2, N], f32, tag="tmp", bufs=1,
                                name="Gt_ps")
                for ic in range(2):
                    # Gt[j, i] = sum_k U[k, j]*diag[k, i] = U[i, j]*rden[i]
                    for jc in range(2):
                        nc.tensor.matmul(
                            Gt_ps[:, jc, 128 * ic:128 * (ic + 1)],
                            U[:, ic, 128 * jc:128 * (jc + 1)],
                            dgs[ic][:], start=True, stop=True)
                Gt_b = sp.tile([128, 2, N], bf16, name="Gt_b", bufs=2)
                nc.vector.tensor_copy(Gt_b[:], Gt_ps[:])

                # aggT[m, i] = sum_j msgs[j, m] * Gt[j, i]
                nc.tensor.matmul(aggT_ps[:, boff:boff + N],
                                 msgs_b[:, 2 * bi, :], Gt_b[:, 0, :],
                                 start=True, stop=False)
                nc.tensor.matmul(aggT_ps[:, boff:boff + N],
                                 msgs_b[:, 2 * bi + 1, :], Gt_b[:, 1, :],
                                 start=False, stop=True)

            aggT_r = aggT_tiles[p % 2]
            nc.vector.tensor_copy(aggT_r[0:MSG, :], aggT_ps[:])

            # receiver MLP
            rT_r = []
            for mi in range(2):
                rps = pp.tile([128, 2 * N], f32, tag="rps", bufs=2,
                              name=f"r{mi}_ps")
                ms = 128 * mi
                nc.tensor.matmul(rps[:], Wr1_r0[:, ms:ms + 128],
                                 od_b[0], start=True, stop=False)
                nc.tensor.matmul(rps[:], Wr1_r1[:, ms:ms + 128],
                                 od_b[1], start=False, stop=False)
                nc.tensor.matmul(rps[:], Wr1c_b[:, ms:ms + 128],
                                 aggT_r[:], start=False, stop=True)
                rr = sp.tile([128, 2 * N], bf16, name=f"r{mi}_r",
                             tag=f"r{mi}r", bufs=2)
                nc.scalar.activation(
                    rr[:], rps[:], AF.Relu,
                    bias=(br1_sb0 if mi == 0 else br1_sb1)[:])
                rT_r.append(rr)

            # output in [d, (b n)] layout
            b0 = 2 * p
            for dc in range(2):
                ds = 128 * dc
                o_ps = pp.tile([128, 2 * N], f32, tag="w2k", bufs=4,
                               name="o_ps")
                nc.tensor.matmul(o_ps[:], Wr2_r0[:, ds:ds + 128],
                                 rT_r[0][:], start=True, stop=False)
                nc.tensor.matmul(o_ps[:], Wr2_r1[:, ds:ds + 128],
                                 rT_r[1][:], start=False, stop=True)
                o_sb = dp.tile([128, 2, N], bf16, name=f"o_sb{dc}",
                               tag=f"osb{dc}", bufs=2)
                nc.scalar.activation(
                    o_sb[:].rearrange("d b n -> d (b n)"), o_ps[:],
                    AF.Copy)
                nc.sync.dma_start(
                    out_d[ds:ds + 128, b0:b0 + 2, :], o_sb[:])

        for _ in range(passes):
            for p in range(npairs):
                emit_pre(p)
                if p > 0:
                    emit_post(p - 1)
            emit_post(npairs - 1)

    nc.compile()
    return nc


def _np_inputs_for_core(inputs, core, bpc=BPC):
    import ml_dtypes

    bf = ml_dtypes.bfloat16
    obs = np.asarray(inputs["obs_all"], np.float32)
    lo = core * bpc
    obsT = np.ascontiguousarray(
        obs[lo:lo + bpc].transpose(2, 0, 1)).astype(bf)

    W1 = np.asarray(inputs["W1"], np.float32)
    W2 = np.asarray(inputs["W2"], np.float32)
    b2 = np.asarray(inputs["b2"], np.float32)
    Wc = np.asarray(inputs["Wc"], np.float32)
    bc = np.asarray(inputs["bc"], np.float32)
    Wd = np.asarray(inputs["Wd"], np.float32)
    bd = np.asarray(inputs["bd"], np.float32)
    Wr1 = np.asarray(inputs["Wr1"], np.float32)

    Wf = (W2 @ Wc) @ Wd                              # [H1, MSG]
    bf_vec = (b2 @ Wc) @ Wd + bc @ Wd + bd           # [MSG]
    Wr1c = Wr1[D:D + MSG]                            # [MSG, H2]
    Wr1c_aug = np.vstack([Wr1c, (bf_vec @ Wr1c)[None, :]])  # [MSG+1, H2]

    return {
        "obsT": obsT,
        "W1": W1.astype(bf),
        "Wf": Wf.astype(bf),
        "Wbil": np.asarray(inputs["Wbil"], np.float32).astype(bf),
        "Wr1a": Wr1[0:D].astype(bf),
        "Wr1c": np.ascontiguousarray(Wr1c_aug).astype(bf),
        "Wr2": np.asarray(inputs["Wr2"], np.float32).astype(bf),
        "b1": np.asarray(inputs["b1"], np.float32),
        "br1": np.asarray(inputs["br1"], np.float32),
    }


def _finish(outT, br2):
    # outT: [D, bpc, N] bf16 -> [bpc, N, D] f32 + br2
    return outT.astype(np.float32).transpose(1, 2, 0) + br2[None, None, :]


def kernel(**inputs):
    from concourse.bass_utils import run_bass_kernel_spmd

    if "prog" not in _CACHE:
        _CACHE["prog"] = build_program(BPC)
    nc = _CACHE["prog"]

    br2 = np.asarray(inputs["br2"], np.float32)
    core_ids = list(range(NCORES))
    in_maps = [_np_inputs_for_core(inputs, c) for c in core_ids]
    res = run_bass_kernel_spmd(nc, in_maps, core_ids)
    out = np.concatenate(
        [_finish(np.asarray(res.results[c]["out"]), br2)
         for c in core_ids], axis=0)
    return out.astype(np.float32)
